# revision 2
# baseline (speedup 1.0000x reference)
"""BEV camera-to-grid scatter kernel for Trainium2 (8 NeuronCores).

Strategy (BEVPoolV2-style):
 - Host (planning only): conservatively cull the 2M frustum points with f64
   geometry + margin (16.6% survive), sort kept points of each camera by their
   target BEV cell, split contiguously across the 8 cores, and pack into
   1408-point blocks whose cell windows (incl. +-1 safety margin) stay under
   512 cells (one PSUM bank). Ship features as bf16 plus small f32 tables
   (pixel coords, depth, per-block affine coefs, per-block exact f32 bin
   thresholds).
 - Device (all 8 cores, SPMD): batched f32 geometry for all points (exact
   reference op structure), per-block exact binning via threshold compares
   diffed into one-hot bin indicators Ax/Ay, scatter-accumulate via bf16
   matmuls (one-hot outer product as moving operand) into per-block PSUM
   windows, accumulated into an SBUF-resident region grid, then AllReduce
   across the 8 cores.
 - Host: paste the reduced region into the (mostly zero) full output.
"""
import sys
import numpy as np

sys.path.insert(0, '/opt/trn_rl_repo')
import ml_dtypes

B, N, D, FH, FW, C = 1, 6, 118, 32, 88, 80
IH, IW = 256, 704
NX, NY, NZ = 360, 360, 1
DXS = (0.3, 0.3, 20.0)
COFF = (-54.0, -54.0, -10.0)   # exact f32 of reference's (bx - dx/2)
NCORES = 8
BLK = 1408                     # points per block: 128 partitions x 11 cols
UJ = 11
MAXW = 512                     # one PSUM bank (f32)
MARGIN_Q = 0.02                # conservative cull margin, in cell units
NCO = 24                       # per-block coefs: A(9) b(3) M(9) t(3)
PADTHR = 3.0e38
f32 = np.float32


# ---------------------------------------------------------------- thresholds
def _thresholds():
    """Exact f32 cell-edge thresholds replicating trunc((g-coff)/dx) binning.

    L[k] = smallest f32 g with q_of(g) >= k (k>=1); L[0] uses q_of(g) > -1
    (reference: trunc + coords>=0 keeps q in (-1,0) in bin 0).
    """
    out = []
    for ax, nb in ((0, NX), (1, NY), (2, NZ)):
        coff = f32(COFF[ax]); dx = f32(DXS[ax])

        def q_of(g):
            return f32(f32(f32(g) - coff) / dx)

        def smallest(pred, lo, hi):
            def key(i):
                return np.int64(i) if i >= 0 else np.int64(-2147483648) - np.int64(i)

            def unkey(k):
                return np.int32(k) if k >= 0 else np.int32(-(k + 2147483648))

            kl = key(f32(lo).view(np.int32)); kh = key(f32(hi).view(np.int32))
            assert not pred(unkey(kl).view(f32)) and pred(unkey(kh).view(f32))
            while kh - kl > 1:
                km = (kl + kh) // 2
                if pred(unkey(km).view(f32)):
                    kh = km
                else:
                    kl = km
            return unkey(kh).view(f32)

        lo_p = f32(coff - 4 * dx); hi_p = f32(coff + (nb + 4) * dx)
        L = np.empty(nb + 1, f32)
        L[0] = smallest(lambda g: q_of(g) > f32(-1.0), lo_p, hi_p)
        for k in range(1, nb + 1):
            L[k] = smallest(lambda g, k=k: q_of(g) >= f32(k), lo_p, hi_p)
        out.append(L)
    return out


_THR_CACHE = []


def _get_thresholds():
    if not _THR_CACHE:
        _THR_CACHE.append(_thresholds())
    return _THR_CACHE[0]


# ------------------------------------------------------------------- planning
def _frustum_axes():
    ds = np.arange(1.0, 60.0, 0.5, dtype=f32)
    xs = np.linspace(0.0, IW - 1, FW, dtype=f32)
    ys = np.linspace(0.0, IH - 1, FH, dtype=f32)
    return ds, xs, ys


def _compute_coeffs(inputs):
    """Fold the reference chain into per-cam affine A,b (pixel->p0) and M,t."""
    aug = np.asarray(inputs['img_aug_matrix'], np.float64)
    c2e = np.asarray(inputs['camera2ego'], np.float64)
    intr = np.asarray(inputs['camera_intrinsics'], np.float64)
    l2e = np.asarray(inputs['lidar2ego'], np.float64)
    laug = np.asarray(inputs['lidar_aug_matrix'], np.float64)
    inv_pr = np.linalg.inv(aug[..., :3, :3])
    post_trans = aug[..., :3, 3]
    A64 = inv_pr
    b64 = -np.einsum('bnij,bnj->bni', inv_pr, post_trans)
    combine = c2e[..., :3, :3] @ np.linalg.inv(intr[..., :3, :3])
    pre = laug[..., :3, :3] @ np.linalg.inv(l2e[..., :3, :3])
    M64 = np.einsum('bij,bnjk->bnik', pre, combine)
    t64 = np.einsum('bij,bnj->bni', pre, c2e[..., :3, 3] - l2e[..., :3, 3][:, None, :]) \
        + laug[..., :3, 3][:, None, :]
    return A64[0], b64[0], M64[0], t64[0]


def _geom64(A, b, M, t, px, py, dv):
    p0 = [A[k, 0] * px + A[k, 1] * py + (A[k, 2] * dv + b[k]) for k in range(3)]
    uu = p0[0] * p0[2]
    vv = p0[1] * p0[2]
    return [(uu * M[k, 0] + vv * M[k, 1]) + p0[2] * M[k, 2] + t[k] for k in range(3)]


def _build_plan(inputs):
    A64, b64, M64, t64 = _compute_coeffs(inputs)
    ds, xs, ys = _frustum_axes()
    dvg, pyg, pxg = np.meshgrid(ds.astype(np.float64), ys.astype(np.float64),
                                xs.astype(np.float64), indexing='ij')
    pxg = np.ascontiguousarray(pxg.ravel())
    pyg = np.ascontiguousarray(pyg.ravel())
    dvg = np.ascontiguousarray(dvg.ravel())
    cams = []
    for n in range(N):
        gx, gy, gz = _geom64(A64[n], b64[n], M64[n], t64[n], pxg, pyg, dvg)
        qx = (gx - COFF[0]) / DXS[0]
        qy = (gy - COFF[1]) / DXS[1]
        qz = (gz - COFF[2]) / DXS[2]
        m = MARGIN_Q
        keep = ((qx > -1 - m) & (qx < NX + m) &
                (qy > -1 - m) & (qy < NY + m) &
                (qz > -1 - m) & (qz < NZ + m))
        idx = np.nonzero(keep)[0]
        kx = np.maximum(np.floor(qx[idx]), 0).astype(np.int64)
        ky = np.maximum(np.floor(qy[idx]), 0).astype(np.int64)
        order = np.argsort(ky * NX + kx, kind='stable')
        cams.append(dict(idx=idx[order], kx=kx[order], ky=ky[order]))
    cores = [[] for _ in range(NCORES)]
    for n in range(N):
        cam = cams[n]
        K = len(cam['idx'])
        bounds = [K * c // NCORES for c in range(NCORES + 1)]
        for c in range(NCORES):
            lo, hi = bounds[c], bounds[c + 1]
            i = lo
            while i < hi:
                j = i
                x0 = x1 = cam['kx'][i]; y0 = y1 = cam['ky'][i]
                while j < hi and j - i < BLK:
                    nx0 = min(x0, cam['kx'][j]); nx1 = max(x1, cam['kx'][j])
                    ny0 = min(y0, cam['ky'][j]); ny1 = max(y1, cam['ky'][j])
                    if (min(int(nx1) + 1, NX - 1) - max(int(nx0) - 1, 0) + 1) * \
                       (min(int(ny1) + 1, NY - 1) - max(int(ny0) - 1, 0) + 1) > MAXW:
                        break
                    x0, x1, y0, y1 = nx0, nx1, ny0, ny1
                    j += 1
                kx0 = max(int(x0) - 1, 0); kx1 = min(int(x1) + 1, NX - 1)
                ky0 = max(int(y0) - 1, 0); ky1 = min(int(y1) + 1, NY - 1)
                cores[c].append(dict(cam=n, idx=cam['idx'][i:j],
                                     kx0=kx0, wx=kx1 - kx0 + 1,
                                     ky0=ky0, wy=ky1 - ky0 + 1))
                i = j
    NBC = max(len(c) for c in cores)
    rx0 = min(b['kx0'] for c in cores for b in c)
    rx1 = max(b['kx0'] + b['wx'] for c in cores for b in c)
    ry0 = min(b['ky0'] for c in cores for b in c)
    ry1 = max(b['ky0'] + b['wy'] for c in cores for b in c)
    Rx, Ry = rx1 - rx0, ry1 - ry0
    # uniform packed-threshold table length across cores
    TX = max(sum(b['wx'] + 1 for b in c) for c in cores)
    TY = max(sum(b['wy'] + 1 for b in c) for c in cores)
    return dict(A64=A64, b64=b64, M64=M64, t64=t64, cores=cores, NBC=NBC,
                rx0=rx0, ry0=ry0, Rx=Rx, Ry=Ry, rcells=Rx * Ry,
                TX=TX, TY=TY, pxg=pxg, pyg=pyg, dvg=dvg)


def _pack_core(plan, inputs, c):
    """Device-side tables for core c."""
    Lx, Ly, Lz = _get_thresholds()
    NBC, TX, TY = plan['NBC'], plan['TX'], plan['TY']
    cf = np.asarray(inputs['cam_feats'], f32)[0].reshape(N, -1, C)
    blocks = plan['cores'][c]
    feats = np.zeros((NBC, BLK, C), ml_dtypes.bfloat16)
    pxt = np.zeros((128, UJ * NBC), f32)
    pyt = np.zeros((128, UJ * NBC), f32)
    dvt = np.zeros((128, UJ * NBC), f32)
    coef = np.zeros((NBC, NCO), f32)
    thrx = np.full((TX,), PADTHR, f32)
    thry = np.full((TY,), PADTHR, f32)
    ox = oy = 0
    meta = []
    for s, blk in enumerate(blocks):
        n = blk['cam']
        idx = blk['idx']
        k = len(idx)
        feats[s, :k] = cf[n][idx].astype(ml_dtypes.bfloat16)
        px = np.zeros(BLK, f32); py = np.zeros(BLK, f32); dv = np.zeros(BLK, f32)
        px[:k] = plan['pxg'][idx]; py[:k] = plan['pyg'][idx]; dv[:k] = plan['dvg'][idx]
        pxt[:, s * UJ:(s + 1) * UJ] = px.reshape(128, UJ)
        pyt[:, s * UJ:(s + 1) * UJ] = py.reshape(128, UJ)
        dvt[:, s * UJ:(s + 1) * UJ] = dv.reshape(128, UJ)
        A = plan['A64'][n].astype(f32); b = plan['b64'][n].astype(f32)
        M = plan['M64'][n].astype(f32); t = plan['t64'][n].astype(f32)
        coef[s] = np.array(list(A.ravel()) + list(b) + list(M.ravel()) + list(t), f32)
        wx, wy = blk['wx'], blk['wy']
        thrx[ox:ox + wx + 1] = Lx[blk['kx0']:blk['kx0'] + wx + 1]
        thry[oy:oy + wy + 1] = Ly[blk['ky0']:blk['ky0'] + wy + 1]
        meta.append(dict(s=s, wx=wx, wy=wy, ox=ox, oy=oy,
                         rxo=blk['kx0'] - plan['rx0'], ryo=blk['ky0'] - plan['ry0']))
        ox += wx + 1; oy += wy + 1
    coefb = np.broadcast_to(coef.reshape(1, NBC * NCO), (128, NBC * NCO)).copy()
    thrxb = np.broadcast_to(thrx.reshape(1, TX), (128, TX)).copy()
    thryb = np.broadcast_to(thry.reshape(1, TY), (128, TY)).copy()
    return dict(feats=feats, pxt=pxt, pyt=pyt, dvt=dvt, coef=coefb,
                thrx=thrxb, thry=thryb), meta


# ----------------------------------------------------------------- bass build
def _build_bass(plan, metas):
    import concourse.bacc as bacc
    import concourse.mybir as mybir
    import concourse.tile as tile

    NBC, TX, TY, rcells = plan['NBC'], plan['TX'], plan['TY'], plan['rcells']
    Rx = plan['Rx']
    SJ = NBC * UJ
    f32t = mybir.dt.float32
    bf16 = mybir.dt.bfloat16
    AL = mybir.AluOpType
    Lx, Ly, Lz = _get_thresholds()
    LZ0, LZ1 = float(Lz[0]), float(Lz[1])

    nc = bacc.Bacc(None, target_bir_lowering=False, num_devices=NCORES)
    feats_t = nc.dram_tensor("feats", [NBC, BLK, C], bf16, kind="ExternalInput")
    pxt_t = nc.dram_tensor("pxt", [128, SJ], f32t, kind="ExternalInput")
    pyt_t = nc.dram_tensor("pyt", [128, SJ], f32t, kind="ExternalInput")
    dvt_t = nc.dram_tensor("dvt", [128, SJ], f32t, kind="ExternalInput")
    coef_t = nc.dram_tensor("coef", [128, NBC * NCO], f32t, kind="ExternalInput")
    thrx_t = nc.dram_tensor("thrx", [128, TX], f32t, kind="ExternalInput")
    thry_t = nc.dram_tensor("thry", [128, TY], f32t, kind="ExternalInput")
    rout_t = nc.dram_tensor("region_out", [C, rcells], f32t, kind="ExternalOutput")

    pid = nc.partition_id()
    maxW = max(m['wx'] * m['wy'] for mm in metas for m in mm)

    with tile.TileContext(nc) as tc:
        with tc.tile_pool(name="tabs", bufs=1) as tp, \
             tc.tile_pool(name="geo", bufs=1) as gp, \
             tc.tile_pool(name="work", bufs=3) as wp, \
             tc.tile_pool(name="oh", bufs=4) as op_, \
             tc.tile_pool(name="ps", bufs=4, space="PSUM") as pp, \
             tc.tile_pool(name="dram", bufs=1, space="DRAM") as dp:

            pxt = tp.tile([128, SJ], f32t); nc.sync.dma_start(pxt[:], pxt_t[:])
            pyt = tp.tile([128, SJ], f32t); nc.sync.dma_start(pyt[:], pyt_t[:])
            dvt = tp.tile([128, SJ], f32t); nc.sync.dma_start(dvt[:], dvt_t[:])
            coef = tp.tile([128, NBC * NCO], f32t); nc.sync.dma_start(coef[:], coef_t[:])
            thrx = tp.tile([128, TX], f32t); nc.sync.dma_start(thrx[:], thrx_t[:])
            thry = tp.tile([128, TY], f32t); nc.sync.dma_start(thry[:], thry_t[:])
            fbuf = tp.tile([128, NBC * UJ * C], bf16)
            fb3 = fbuf[:].rearrange("p (s x) -> p s x", x=UJ * C)
            fsrc = feats_t[:].rearrange("s (p j) c -> p s (j c)", p=128)
            half = NBC // 2
            nc.sync.dma_start(fb3[:, :half, :], fsrc[:, :half, :])
            nc.sync.dma_start(fb3[:, half:, :], fsrc[:, half:, :])

            region = gp.tile([C, rcells], f32t)
            nc.vector.memset(region[:], 0.0)

            def cslice(kidx):
                ap = coef[:].rearrange("p (s k) -> p s k", k=NCO)[:, :, kidx:kidx + 1]
                return ap.broadcast_to([128, NBC, UJ])

            def g3(ap):
                return ap.rearrange("p (s j) -> p s j", j=UJ)

            # ---- batched geometry, exact f32 op order ----
            tmpa = gp.tile([128, SJ], f32t)
            tmpb = gp.tile([128, SJ], f32t)
            p0 = [gp.tile([128, SJ], f32t, name=f'p0_{i}', tag=f'p0_{i}')
                  for i in range(3)]
            for kk in range(3):
                nc.vector.tensor_tensor(out=g3(tmpa[:]), in0=g3(pxt[:]),
                                        in1=cslice(3 * kk + 0), op=AL.mult)
                nc.vector.tensor_tensor(out=g3(tmpb[:]), in0=g3(pyt[:]),
                                        in1=cslice(3 * kk + 1), op=AL.mult)
                nc.vector.tensor_tensor(out=tmpa[:], in0=tmpa[:], in1=tmpb[:], op=AL.add)
                nc.vector.tensor_tensor(out=g3(tmpb[:]), in0=g3(dvt[:]),
                                        in1=cslice(3 * kk + 2), op=AL.mult)
                nc.vector.tensor_tensor(out=g3(tmpb[:]), in0=g3(tmpb[:]),
                                        in1=cslice(9 + kk), op=AL.add)
                nc.vector.tensor_tensor(out=p0[kk][:], in0=tmpa[:], in1=tmpb[:], op=AL.add)
            uu = gp.tile([128, SJ], f32t)
            vv = gp.tile([128, SJ], f32t)
            nc.vector.tensor_tensor(out=uu[:], in0=p0[0][:], in1=p0[2][:], op=AL.mult)
            nc.vector.tensor_tensor(out=vv[:], in0=p0[1][:], in1=p0[2][:], op=AL.mult)
            g = [gp.tile([128, SJ], f32t, name=f'g_{i}', tag=f'g_{i}') for i in range(3)]
            for kk in range(3):
                base = 12 + 3 * kk
                nc.vector.tensor_tensor(out=g3(tmpa[:]), in0=g3(uu[:]),
                                        in1=cslice(base + 0), op=AL.mult)
                nc.vector.tensor_tensor(out=g3(tmpb[:]), in0=g3(vv[:]),
                                        in1=cslice(base + 1), op=AL.mult)
                nc.vector.tensor_tensor(out=tmpa[:], in0=tmpa[:], in1=tmpb[:], op=AL.add)
                nc.vector.tensor_tensor(out=g3(tmpb[:]), in0=g3(p0[2][:]),
                                        in1=cslice(base + 2), op=AL.mult)
                nc.vector.tensor_tensor(out=tmpa[:], in0=tmpa[:], in1=tmpb[:], op=AL.add)
                nc.vector.tensor_tensor(out=g3(g[kk][:]), in0=g3(tmpa[:]),
                                        in1=cslice(21 + kk), op=AL.add)
            gx, gy, gz = g
            # ---- z-range mask (NZ=1): zm = (gz >= Lz0) * (gz < Lz1) ----
            zm = gp.tile([128, SJ], f32t)
            nc.vector.tensor_scalar(out=tmpa[:], in0=gz[:], scalar1=LZ0,
                                    scalar2=None, op0=AL.is_ge)
            nc.vector.tensor_scalar(out=tmpb[:], in0=gz[:], scalar1=LZ1,
                                    scalar2=None, op0=AL.is_lt)
            nc.vector.tensor_tensor(out=zm[:], in0=tmpa[:], in1=tmpb[:], op=AL.mult)

            region3 = region[:].rearrange("p (y x) -> p y x", x=Rx)
            gx3 = g3(gx[:]); gy3 = g3(gy[:]); zm3 = g3(zm[:])

            # ---- per-core sections ----
            for core_id in range(NCORES):
                mm = metas[core_id]
                with tc.If(pid == core_id):
                    for m in mm:
                        s, wx, wy = m['s'], m['wx'], m['wy']
                        W = wx * wy
                        cx = wp.tile([128, UJ * (wx + 1)], f32t, tag="cx")
                        cx3 = cx[:].rearrange("p (j w) -> p j w", w=wx + 1)
                        nc.vector.tensor_tensor(
                            out=cx3,
                            in0=gx3[:, s, :, None].broadcast_to([128, UJ, wx + 1]),
                            in1=thrx[:, None, m['ox']:m['ox'] + wx + 1]
                                .broadcast_to([128, UJ, wx + 1]),
                            op=AL.is_ge)
                        ax = wp.tile([128, UJ * wx], bf16, tag="ax")
                        nc.vector.tensor_tensor(
                            out=ax[:].rearrange("p (j w) -> p j w", w=wx),
                            in0=cx3[:, :, 0:wx], in1=cx3[:, :, 1:wx + 1],
                            op=AL.subtract)
                        cy = wp.tile([128, UJ * (wy + 1)], f32t, tag="cy")
                        cy3 = cy[:].rearrange("p (j w) -> p j w", w=wy + 1)
                        nc.vector.tensor_tensor(
                            out=cy3,
                            in0=gy3[:, s, :, None].broadcast_to([128, UJ, wy + 1]),
                            in1=thry[:, None, m['oy']:m['oy'] + wy + 1]
                                .broadcast_to([128, UJ, wy + 1]),
                            op=AL.is_ge)
                        ayf = wp.tile([128, UJ * wy], f32t, tag="ayf")
                        ayf3 = ayf[:].rearrange("p (j w) -> p j w", w=wy)
                        nc.vector.tensor_tensor(
                            out=ayf3, in0=cy3[:, :, 0:wy], in1=cy3[:, :, 1:wy + 1],
                            op=AL.subtract)
                        ay = wp.tile([128, UJ * wy], bf16, tag="ay")
                        nc.vector.tensor_tensor(
                            out=ay[:].rearrange("p (j w) -> p j w", w=wy),
                            in0=ayf3,
                            in1=zm3[:, s, :, None].broadcast_to([128, UJ, wy]),
                            op=AL.mult)
                        oh = op_.tile([128, UJ * maxW], bf16, tag="oh")
                        oh4 = oh[:, :UJ * W].rearrange("p (j y x) -> p j y x", y=wy, x=wx)
                        nc.vector.tensor_tensor(
                            out=oh4,
                            in0=ay[:].rearrange("p (j y) -> p j y", y=wy)[:, :, :, None]
                                .broadcast_to([128, UJ, wy, wx]),
                            in1=ax[:].rearrange("p (j x) -> p j x", x=wx)[:, :, None, :]
                                .broadcast_to([128, UJ, wy, wx]),
                            op=AL.mult)
                        ps = pp.tile([C, maxW], mybir.dt.float32, space="PSUM", tag="ps")
                        for j in range(UJ):
                            nc.tensor.matmul(
                                ps[:, :W],
                                lhsT=fbuf[:, (s * UJ + j) * C:(s * UJ + j + 1) * C],
                                rhs=oh[:, j * W:(j + 1) * W],
                                start=(j == 0), stop=(j == UJ - 1))
                        dst = region3[:, m['ryo']:m['ryo'] + wy, m['rxo']:m['rxo'] + wx]
                        nc.vector.tensor_tensor(
                            out=dst, in0=dst,
                            in1=ps[:, :W].rearrange("p (y x) -> p y x", x=wx),
                            op=AL.add)

            # ---- epilogue: allreduce partial regions ----
            rpart = dp.tile([C, rcells], f32t)
            rsum = dp.tile([C, rcells], f32t)
            nc.sync.dma_start(rpart[:], region[:])
            nc.gpsimd.collective_compute(
                "AllReduce", AL.add,
                replica_groups=[list(range(NCORES))],
                ins=[rpart[:]], outs=[rsum[:]])
            nc.sync.dma_start(rout_t[:], rsum[:])

    nc.compile()
    return nc


_CACHE = {}


def _plan_key(metas, plan):
    return (plan['NBC'], plan['TX'], plan['TY'], plan['rcells'], plan['Rx'],
            tuple(tuple((m['wx'], m['wy'], m['ox'], m['oy'], m['rxo'], m['ryo'])
                        for m in mm) for mm in metas))


def kernel(**inputs) -> np.ndarray:
    from concourse.bass_utils import run_bass_kernel_spmd

    plan = _build_plan(inputs)
    packed = [_pack_core(plan, inputs, c) for c in range(NCORES)]
    in_maps = [p[0] for p in packed]
    metas = [p[1] for p in packed]
    key = _plan_key(metas, plan)
    if key not in _CACHE:
        _CACHE.clear()
        _CACHE[key] = _build_bass(plan, metas)
    nc = _CACHE[key]

    r = run_bass_kernel_spmd(nc, in_maps, core_ids=list(range(NCORES)))
    region = r.results[0]['region_out']          # [C, rcells] summed over cores
    out = np.zeros((B, C, NX, NY), f32)
    Rx, Ry = plan['Rx'], plan['Ry']
    blk = region.reshape(C, Ry, Rx).transpose(0, 2, 1)
    out[0, :, plan['rx0']:plan['rx0'] + Rx, plan['ry0']:plan['ry0'] + Ry] = blk
    return out


# revision 4
# speedup vs baseline: 20.8567x; 20.8567x over previous
"""BEV camera-to-grid scatter kernel for Trainium2 (8 NeuronCores).

Strategy (BEVPoolV2-style):
 - Host (planning only): conservatively cull the 2M frustum points with f64
   geometry + margin (16.6% survive), sort kept points of each camera along a
   Morton curve of their target BEV cell, split contiguously across the 8
   cores, and pack into 1408-point blocks whose (margin-padded) cell windows
   fit a uniform WXP x WYP class. Ship features as bf16 plus small f32 tables
   (pixel coords, depth, per-block affine coefs, per-block exact f32 bin-edge
   thresholds).
 - Device (one uniform SPMD program, no control flow): batched f32 geometry
   for all points (exact reference op structure), exact binning via threshold
   compares diffed into per-axis one-hot indicators Ax/Ay, per-block one-hot
   outer products, bf16 matmul scatter into per-block PSUM windows, results
   appended to a per-block slots buffer, one DMA out. Cores are pure data
   parallel - no cross-core communication on device.
 - Host (unshard): place each block's window into the full (mostly zero) BEV
   grid and sum across cores (scatter-add is associative).
"""
import sys
import numpy as np

sys.path.insert(0, '/opt/trn_rl_repo')
import ml_dtypes

B, N, D, FH, FW, C = 1, 6, 118, 32, 88, 80
IH, IW = 256, 704
NX, NY, NZ = 360, 360, 1
DXS = (0.3, 0.3, 20.0)
COFF = (-54.0, -54.0, -10.0)   # exact f32 of reference's (bx - dx/2)
NCORES = 8
BLK = 1408                     # points per block: 128 partitions x 11 cols
UJ = 11
WXP, WYP = 16, 10              # uniform per-block window class
WP = WXP * WYP
MARGIN_Q = 0.02                # conservative cull margin, in cell units
NCO = 24                       # per-block coefs: A(9) b(3) M(9) t(3)
PADTHR = 3.0e38
f32 = np.float32


# ---------------------------------------------------------------- thresholds
def _thresholds():
    """Exact f32 cell-edge thresholds replicating trunc((g-coff)/dx) binning.

    L[k] = smallest f32 g with q_of(g) >= k (k>=1); L[0] uses q_of(g) > -1
    (reference: trunc + coords>=0 keeps q in (-1,0) in bin 0).
    """
    out = []
    for ax, nb in ((0, NX), (1, NY), (2, NZ)):
        coff = f32(COFF[ax]); dx = f32(DXS[ax])

        def q_of(g):
            return f32(f32(f32(g) - coff) / dx)

        def smallest(pred, lo, hi):
            def key(i):
                return np.int64(i) if i >= 0 else np.int64(-2147483648) - np.int64(i)

            def unkey(k):
                return np.int32(k) if k >= 0 else np.int32(-(k + 2147483648))

            kl = key(f32(lo).view(np.int32)); kh = key(f32(hi).view(np.int32))
            assert not pred(unkey(kl).view(f32)) and pred(unkey(kh).view(f32))
            while kh - kl > 1:
                km = (kl + kh) // 2
                if pred(unkey(km).view(f32)):
                    kh = km
                else:
                    kl = km
            return unkey(kh).view(f32)

        lo_p = f32(coff - 4 * dx); hi_p = f32(coff + (nb + 4) * dx)
        L = np.empty(nb + 1, f32)
        L[0] = smallest(lambda g: q_of(g) > f32(-1.0), lo_p, hi_p)
        for k in range(1, nb + 1):
            L[k] = smallest(lambda g, k=k: q_of(g) >= f32(k), lo_p, hi_p)
        out.append(L)
    return out


_THR_CACHE = []


def _get_thresholds():
    if not _THR_CACHE:
        _THR_CACHE.append(_thresholds())
    return _THR_CACHE[0]


# ------------------------------------------------------------------- planning
def _frustum_axes():
    ds = np.arange(1.0, 60.0, 0.5, dtype=f32)
    xs = np.linspace(0.0, IW - 1, FW, dtype=f32)
    ys = np.linspace(0.0, IH - 1, FH, dtype=f32)
    return ds, xs, ys


def _compute_coeffs(inputs):
    """Fold the reference chain into per-cam affine A,b (pixel->p0) and M,t."""
    aug = np.asarray(inputs['img_aug_matrix'], np.float64)
    c2e = np.asarray(inputs['camera2ego'], np.float64)
    intr = np.asarray(inputs['camera_intrinsics'], np.float64)
    l2e = np.asarray(inputs['lidar2ego'], np.float64)
    laug = np.asarray(inputs['lidar_aug_matrix'], np.float64)
    inv_pr = np.linalg.inv(aug[..., :3, :3])
    post_trans = aug[..., :3, 3]
    A64 = inv_pr
    b64 = -np.einsum('bnij,bnj->bni', inv_pr, post_trans)
    combine = c2e[..., :3, :3] @ np.linalg.inv(intr[..., :3, :3])
    pre = laug[..., :3, :3] @ np.linalg.inv(l2e[..., :3, :3])
    M64 = np.einsum('bij,bnjk->bnik', pre, combine)
    t64 = np.einsum('bij,bnj->bni', pre, c2e[..., :3, 3] - l2e[..., :3, 3][:, None, :]) \
        + laug[..., :3, 3][:, None, :]
    return A64[0], b64[0], M64[0], t64[0]


def _geom64(A, b, M, t, px, py, dv):
    p0 = [A[k, 0] * px + A[k, 1] * py + (A[k, 2] * dv + b[k]) for k in range(3)]
    uu = p0[0] * p0[2]
    vv = p0[1] * p0[2]
    return [(uu * M[k, 0] + vv * M[k, 1]) + p0[2] * M[k, 2] + t[k] for k in range(3)]


def _build_plan(inputs):
    A64, b64, M64, t64 = _compute_coeffs(inputs)
    ds, xs, ys = _frustum_axes()
    dvg, pyg, pxg = np.meshgrid(ds.astype(np.float64), ys.astype(np.float64),
                                xs.astype(np.float64), indexing='ij')
    pxg = np.ascontiguousarray(pxg.ravel())
    pyg = np.ascontiguousarray(pyg.ravel())
    dvg = np.ascontiguousarray(dvg.ravel())
    cores = [[] for _ in range(NCORES)]
    for n in range(N):
        gx, gy, gz = _geom64(A64[n], b64[n], M64[n], t64[n], pxg, pyg, dvg)
        qx = (gx - COFF[0]) / DXS[0]
        qy = (gy - COFF[1]) / DXS[1]
        qz = (gz - COFF[2]) / DXS[2]
        m = MARGIN_Q
        keep = ((qx > -1 - m) & (qx < NX + m) &
                (qy > -1 - m) & (qy < NY + m) &
                (qz > -1 - m) & (qz < NZ + m))
        idx = np.nonzero(keep)[0]
        kx = np.maximum(np.floor(qx[idx]), 0).astype(np.int64)
        ky = np.maximum(np.floor(qy[idx]), 0).astype(np.int64)
        code = np.zeros(len(kx), np.int64)
        for bit in range(9):
            code |= ((kx >> bit) & 1) << (2 * bit) | ((ky >> bit) & 1) << (2 * bit + 1)
        order = np.argsort(code, kind='stable')
        idx, kx, ky = idx[order], kx[order], ky[order]
        K = len(idx)
        bounds = [K * c // NCORES for c in range(NCORES + 1)]
        for c in range(NCORES):
            lo, hi = bounds[c], bounds[c + 1]
            i = lo
            while i < hi:
                j = i
                x0 = x1 = kx[i]; y0 = y1 = ky[i]
                while j < hi and j - i < BLK:
                    nx0 = min(x0, kx[j]); nx1 = max(x1, kx[j])
                    ny0 = min(y0, ky[j]); ny1 = max(y1, ky[j])
                    if nx1 - nx0 + 3 > WXP or ny1 - ny0 + 3 > WYP:
                        break
                    x0, x1, y0, y1 = nx0, nx1, ny0, ny1
                    j += 1
                cores[c].append(dict(idx=idx[i:j], cam=n,
                                     kx0=max(int(x0) - 1, 0),
                                     ky0=max(int(y0) - 1, 0)))
                i = j
    NBC = max(len(c) for c in cores)
    return dict(A64=A64, b64=b64, M64=M64, t64=t64, cores=cores, NBC=NBC,
                pxg=pxg, pyg=pyg, dvg=dvg)


def _pack_core(plan, inputs, c):
    """Device-side tables for core c."""
    Lx, Ly, Lz = _get_thresholds()
    NBC = plan['NBC']
    cf = np.asarray(inputs['cam_feats'], f32)[0].reshape(N, -1, C)
    blocks = plan['cores'][c]
    feats = np.zeros((NBC, BLK, C), ml_dtypes.bfloat16)
    pxt = np.zeros((128, UJ * NBC), f32)
    pyt = np.zeros((128, UJ * NBC), f32)
    dvt = np.zeros((128, UJ * NBC), f32)
    coef = np.zeros((NBC, NCO), f32)
    thrx = np.full((NBC, WXP + 1), PADTHR, f32)
    thry = np.full((NBC, WYP + 1), PADTHR, f32)
    for s, blk in enumerate(blocks):
        n = blk['cam']
        idx = blk['idx']
        k = len(idx)
        feats[s, :k] = cf[n][idx].astype(ml_dtypes.bfloat16)
        px = np.zeros(BLK, f32); py = np.zeros(BLK, f32); dv = np.zeros(BLK, f32)
        px[:k] = plan['pxg'][idx]; py[:k] = plan['pyg'][idx]; dv[:k] = plan['dvg'][idx]
        pxt[:, s * UJ:(s + 1) * UJ] = px.reshape(128, UJ)
        pyt[:, s * UJ:(s + 1) * UJ] = py.reshape(128, UJ)
        dvt[:, s * UJ:(s + 1) * UJ] = dv.reshape(128, UJ)
        A = plan['A64'][n].astype(f32); b = plan['b64'][n].astype(f32)
        M = plan['M64'][n].astype(f32); t = plan['t64'][n].astype(f32)
        coef[s] = np.array(list(A.ravel()) + list(b) + list(M.ravel()) + list(t), f32)
        ex = min(WXP + 1, NX + 1 - blk['kx0'])
        ey = min(WYP + 1, NY + 1 - blk['ky0'])
        thrx[s, :ex] = Lx[blk['kx0']:blk['kx0'] + ex]
        thry[s, :ey] = Ly[blk['ky0']:blk['ky0'] + ey]
    coefb = np.broadcast_to(coef.reshape(1, NBC * NCO), (128, NBC * NCO)).copy()
    thrxb = np.broadcast_to(thrx.reshape(1, NBC * (WXP + 1)),
                            (128, NBC * (WXP + 1))).copy()
    thryb = np.broadcast_to(thry.reshape(1, NBC * (WYP + 1)),
                            (128, NBC * (WYP + 1))).copy()
    return dict(feats=feats, pxt=pxt, pyt=pyt, dvt=dvt, coef=coefb,
                thrx=thrxb, thry=thryb)


# ----------------------------------------------------------------- bass build
def _build_bass(NBC):
    import concourse.bacc as bacc
    import concourse.mybir as mybir
    import concourse.tile as tile

    SJ = NBC * UJ
    f32t = mybir.dt.float32
    bf16 = mybir.dt.bfloat16
    AL = mybir.AluOpType
    Lx, Ly, Lz = _get_thresholds()
    LZ0, LZ1 = float(Lz[0]), float(Lz[1])

    nc = bacc.Bacc(None, target_bir_lowering=False, num_devices=NCORES)
    feats_t = nc.dram_tensor("feats", [NBC, BLK, C], bf16, kind="ExternalInput")
    pxt_t = nc.dram_tensor("pxt", [128, SJ], f32t, kind="ExternalInput")
    pyt_t = nc.dram_tensor("pyt", [128, SJ], f32t, kind="ExternalInput")
    dvt_t = nc.dram_tensor("dvt", [128, SJ], f32t, kind="ExternalInput")
    coef_t = nc.dram_tensor("coef", [128, NBC * NCO], f32t, kind="ExternalInput")
    thrx_t = nc.dram_tensor("thrx", [128, NBC * (WXP + 1)], f32t, kind="ExternalInput")
    thry_t = nc.dram_tensor("thry", [128, NBC * (WYP + 1)], f32t, kind="ExternalInput")
    slots_t = nc.dram_tensor("slots", [C, NBC * WP], f32t, kind="ExternalOutput")

    with tile.TileContext(nc) as tc:
        with tc.tile_pool(name="tabs", bufs=1) as tp, \
             tc.tile_pool(name="fb", bufs=4) as fp, \
             tc.tile_pool(name="oh", bufs=4) as op_, \
             tc.tile_pool(name="ps", bufs=6, space="PSUM") as pp:

            pxt = tp.tile([128, SJ], f32t); nc.sync.dma_start(pxt[:], pxt_t[:])
            pyt = tp.tile([128, SJ], f32t); nc.sync.dma_start(pyt[:], pyt_t[:])
            dvt = tp.tile([128, SJ], f32t); nc.sync.dma_start(dvt[:], dvt_t[:])
            coef = tp.tile([128, NBC * NCO], f32t); nc.sync.dma_start(coef[:], coef_t[:])
            thrx = tp.tile([128, NBC * (WXP + 1)], f32t)
            nc.sync.dma_start(thrx[:], thrx_t[:])
            thry = tp.tile([128, NBC * (WYP + 1)], f32t)
            nc.sync.dma_start(thry[:], thry_t[:])

            def cslice(kidx):
                ap = coef[:].rearrange("p (s k) -> p s k", k=NCO)[:, :, kidx:kidx + 1]
                return ap.broadcast_to([128, NBC, UJ])

            def g3(ap):
                return ap.rearrange("p (s j) -> p s j", j=UJ)

            # ---- batched geometry, exact f32 op order ----
            tmpa = tp.tile([128, SJ], f32t)
            tmpb = tp.tile([128, SJ], f32t)
            p0 = [tp.tile([128, SJ], f32t, name=f'p0_{i}', tag=f'p0_{i}')
                  for i in range(3)]
            for kk in range(3):
                nc.vector.tensor_tensor(out=g3(tmpa[:]), in0=g3(pxt[:]),
                                        in1=cslice(3 * kk + 0), op=AL.mult)
                nc.vector.tensor_tensor(out=g3(tmpb[:]), in0=g3(pyt[:]),
                                        in1=cslice(3 * kk + 1), op=AL.mult)
                nc.vector.tensor_tensor(out=tmpa[:], in0=tmpa[:], in1=tmpb[:], op=AL.add)
                nc.vector.tensor_tensor(out=g3(tmpb[:]), in0=g3(dvt[:]),
                                        in1=cslice(3 * kk + 2), op=AL.mult)
                nc.vector.tensor_tensor(out=g3(tmpb[:]), in0=g3(tmpb[:]),
                                        in1=cslice(9 + kk), op=AL.add)
                nc.vector.tensor_tensor(out=p0[kk][:], in0=tmpa[:], in1=tmpb[:], op=AL.add)
            uu = tp.tile([128, SJ], f32t)
            vv = tp.tile([128, SJ], f32t)
            nc.vector.tensor_tensor(out=uu[:], in0=p0[0][:], in1=p0[2][:], op=AL.mult)
            nc.vector.tensor_tensor(out=vv[:], in0=p0[1][:], in1=p0[2][:], op=AL.mult)
            g = [tp.tile([128, SJ], f32t, name=f'g_{i}', tag=f'g_{i}') for i in range(3)]
            for kk in range(3):
                base = 12 + 3 * kk
                nc.vector.tensor_tensor(out=g3(tmpa[:]), in0=g3(uu[:]),
                                        in1=cslice(base + 0), op=AL.mult)
                nc.vector.tensor_tensor(out=g3(tmpb[:]), in0=g3(vv[:]),
                                        in1=cslice(base + 1), op=AL.mult)
                nc.vector.tensor_tensor(out=tmpa[:], in0=tmpa[:], in1=tmpb[:], op=AL.add)
                nc.vector.tensor_tensor(out=g3(tmpb[:]), in0=g3(p0[2][:]),
                                        in1=cslice(base + 2), op=AL.mult)
                nc.vector.tensor_tensor(out=tmpa[:], in0=tmpa[:], in1=tmpb[:], op=AL.add)
                nc.vector.tensor_tensor(out=g3(g[kk][:]), in0=g3(tmpa[:]),
                                        in1=cslice(21 + kk), op=AL.add)
            gx, gy, gz = g
            # ---- z-range mask (NZ=1): zm = (gz >= Lz0) * (gz < Lz1) ----
            zm = tp.tile([128, SJ], f32t)
            nc.vector.tensor_scalar(out=tmpa[:], in0=gz[:], scalar1=LZ0,
                                    scalar2=None, op0=AL.is_ge)
            nc.vector.tensor_scalar(out=tmpb[:], in0=gz[:], scalar1=LZ1,
                                    scalar2=None, op0=AL.is_lt)
            nc.vector.tensor_tensor(out=zm[:], in0=tmpa[:], in1=tmpb[:], op=AL.mult)

            gx4 = gx[:].rearrange("p (s j) -> p s j", j=UJ)
            gy4 = gy[:].rearrange("p (s j) -> p s j", j=UJ)
            zm4 = zm[:].rearrange("p (s j) -> p s j", j=UJ)

            # ---- batched exact binning: per-axis one-hot indicators ----
            WX1, WY1 = WXP + 1, WYP + 1
            cxa = tp.tile([128, NBC * UJ * WX1], bf16)
            cxa4 = cxa[:].rearrange("p (s j w) -> p s j w", j=UJ, w=WX1)
            nc.vector.tensor_tensor(
                out=cxa4,
                in0=gx4[:, :, :, None].broadcast_to([128, NBC, UJ, WX1]),
                in1=thrx[:].rearrange("p (s w) -> p s w", w=WX1)[:, :, None, :]
                    .broadcast_to([128, NBC, UJ, WX1]),
                op=AL.is_ge)
            axa = tp.tile([128, NBC * UJ * WXP], bf16)
            axa4 = axa[:].rearrange("p (s j w) -> p s j w", j=UJ, w=WXP)
            nc.vector.tensor_tensor(out=axa4, in0=cxa4[:, :, :, 0:WXP],
                                    in1=cxa4[:, :, :, 1:WX1], op=AL.subtract)
            cya = tp.tile([128, NBC * UJ * WY1], bf16)
            cya4 = cya[:].rearrange("p (s j w) -> p s j w", j=UJ, w=WY1)
            nc.vector.tensor_tensor(
                out=cya4,
                in0=gy4[:, :, :, None].broadcast_to([128, NBC, UJ, WY1]),
                in1=thry[:].rearrange("p (s w) -> p s w", w=WY1)[:, :, None, :]
                    .broadcast_to([128, NBC, UJ, WY1]),
                op=AL.is_ge)
            ayt = tp.tile([128, NBC * UJ * WYP], bf16)
            ayt4 = ayt[:].rearrange("p (s j w) -> p s j w", j=UJ, w=WYP)
            nc.vector.tensor_tensor(out=ayt4, in0=cya4[:, :, :, 0:WYP],
                                    in1=cya4[:, :, :, 1:WY1], op=AL.subtract)
            nc.vector.tensor_tensor(
                out=ayt4, in0=ayt4,
                in1=zm4[:, :, :, None].broadcast_to([128, NBC, UJ, WYP]),
                op=AL.mult)

            slots = tp.tile([C, NBC * WP], f32t)

            # ---- per-block: one-hot outer product + matmul scatter ----
            PAIR = 2
            for s0 in range(0, NBC, PAIR):
                sl = min(PAIR, NBC - s0)
                fb = fp.tile([128, PAIR * UJ * C], bf16, tag="fb", name="fb")
                nc.sync.dma_start(
                    fb[:, :sl * UJ * C].rearrange("p (s x) -> p s x", x=UJ * C),
                    feats_t[s0:s0 + sl].rearrange("s (p j) c -> p s (j c)", p=128))
                for si in range(sl):
                    s = s0 + si
                    oh = op_.tile([128, UJ * WP], bf16, tag="oh", name="oh")
                    oh4 = oh[:].rearrange("p (j y x) -> p j y x", y=WYP, x=WXP)
                    nc.vector.tensor_tensor(
                        out=oh4,
                        in0=ayt4[:, s][:, :, :, None].broadcast_to([128, UJ, WYP, WXP]),
                        in1=axa4[:, s][:, :, None, :].broadcast_to([128, UJ, WYP, WXP]),
                        op=AL.mult)
                    ps = pp.tile([C, WP], mybir.dt.float32, space="PSUM", tag="ps",
                                 name="ps")
                    for j in range(UJ):
                        nc.tensor.matmul(
                            ps[:],
                            lhsT=fb[:, (si * UJ + j) * C:(si * UJ + j + 1) * C],
                            rhs=oh[:, j * WP:(j + 1) * WP],
                            start=(j == 0), stop=(j == UJ - 1))
                    nc.vector.tensor_copy(out=slots[:, s * WP:(s + 1) * WP], in_=ps[:])

            nc.sync.dma_start(slots_t[:], slots[:])

    nc.compile()
    return nc


_CACHE = {}


def kernel(**inputs) -> np.ndarray:
    from concourse.bass_utils import run_bass_kernel_spmd

    plan = _build_plan(inputs)
    in_maps = [_pack_core(plan, inputs, c) for c in range(NCORES)]
    NBC = plan['NBC']
    if NBC not in _CACHE:
        _CACHE.clear()
        _CACHE[NBC] = _build_bass(NBC)
    nc = _CACHE[NBC]

    r = run_bass_kernel_spmd(nc, in_maps, core_ids=list(range(NCORES)))
    out = np.zeros((B, C, NX, NY), f32)
    for c in range(NCORES):
        slots = r.results[c]['slots'].reshape(C, NBC, WYP, WXP)
        for s, blk in enumerate(plan['cores'][c]):
            kx0, ky0 = blk['kx0'], blk['ky0']
            ex = min(WXP, NX - kx0); ey = min(WYP, NY - ky0)
            # slot layout [C, y, x]; output layout [C, X, Y]
            out[0, :, kx0:kx0 + ex, ky0:ky0 + ey] += \
                slots[:, s, :ey, :ex].transpose(0, 2, 1)
    return out


# revision 6
# speedup vs baseline: 34.3389x; 1.6464x over previous
"""BEV camera-to-grid scatter kernel for Trainium2 (8 NeuronCores).

Strategy (BEVPoolV2-style):
 - Host (planning only): conservatively cull the 2M frustum points with f64
   geometry + margin (16.6% survive), sort kept points of each camera along a
   Morton curve of their target BEV cell, split contiguously across the 8
   cores, and pack into 1408-point blocks whose (margin-padded) cell windows
   fit a uniform WXP x WYP class. Ship features as bf16 plus small f32 tables
   (pixel coords, depth, per-block affine coefs, per-block exact f32 bin-edge
   thresholds).
 - Device (one uniform SPMD program, no control flow): batched f32 geometry
   for all points (exact reference op structure), exact binning via threshold
   compares diffed into per-axis one-hot indicators Ax/Ay, per-block one-hot
   outer products, bf16 matmul scatter into per-block PSUM windows, results
   appended to a per-block slots buffer, one DMA out. Cores are pure data
   parallel - no cross-core communication on device.
 - Host (unshard): place each block's window into the full (mostly zero) BEV
   grid and sum across cores (scatter-add is associative).
"""
import sys
import numpy as np

sys.path.insert(0, '/opt/trn_rl_repo')
import ml_dtypes

B, N, D, FH, FW, C = 1, 6, 118, 32, 88, 80
IH, IW = 256, 704
NX, NY, NZ = 360, 360, 1
DXS = (0.3, 0.3, 20.0)
COFF = (-54.0, -54.0, -10.0)   # exact f32 of reference's (bx - dx/2)
NCORES = 8
BLK = 1408                     # points per block: 128 partitions x 11 cols
UJ = 11
WXP, WYP = 16, 10              # uniform per-block window class
WP = WXP * WYP
MARGIN_Q = 0.02                # conservative cull margin, in cell units
NCO = 24                       # per-block coefs: A(9) b(3) M(9) t(3)
PADTHR = 3.0e38
f32 = np.float32


# ---------------------------------------------------------------- thresholds
def _thresholds():
    """Exact f32 cell-edge thresholds replicating trunc((g-coff)/dx) binning.

    L[k] = smallest f32 g with q_of(g) >= k (k>=1); L[0] uses q_of(g) > -1
    (reference: trunc + coords>=0 keeps q in (-1,0) in bin 0).
    """
    out = []
    for ax, nb in ((0, NX), (1, NY), (2, NZ)):
        coff = f32(COFF[ax]); dx = f32(DXS[ax])

        def q_of(g):
            return f32(f32(f32(g) - coff) / dx)

        def smallest(pred, lo, hi):
            def key(i):
                return np.int64(i) if i >= 0 else np.int64(-2147483648) - np.int64(i)

            def unkey(k):
                return np.int32(k) if k >= 0 else np.int32(-(k + 2147483648))

            kl = key(f32(lo).view(np.int32)); kh = key(f32(hi).view(np.int32))
            assert not pred(unkey(kl).view(f32)) and pred(unkey(kh).view(f32))
            while kh - kl > 1:
                km = (kl + kh) // 2
                if pred(unkey(km).view(f32)):
                    kh = km
                else:
                    kl = km
            return unkey(kh).view(f32)

        lo_p = f32(coff - 4 * dx); hi_p = f32(coff + (nb + 4) * dx)
        L = np.empty(nb + 1, f32)
        L[0] = smallest(lambda g: q_of(g) > f32(-1.0), lo_p, hi_p)
        for k in range(1, nb + 1):
            L[k] = smallest(lambda g, k=k: q_of(g) >= f32(k), lo_p, hi_p)
        out.append(L)
    return out


_THR_CACHE = []


def _get_thresholds():
    if not _THR_CACHE:
        _THR_CACHE.append(_thresholds())
    return _THR_CACHE[0]


# ------------------------------------------------------------------- planning
def _frustum_axes():
    ds = np.arange(1.0, 60.0, 0.5, dtype=f32)
    xs = np.linspace(0.0, IW - 1, FW, dtype=f32)
    ys = np.linspace(0.0, IH - 1, FH, dtype=f32)
    return ds, xs, ys


def _compute_coeffs(inputs):
    """Fold the reference chain into per-cam affine A,b (pixel->p0) and M,t."""
    aug = np.asarray(inputs['img_aug_matrix'], np.float64)
    c2e = np.asarray(inputs['camera2ego'], np.float64)
    intr = np.asarray(inputs['camera_intrinsics'], np.float64)
    l2e = np.asarray(inputs['lidar2ego'], np.float64)
    laug = np.asarray(inputs['lidar_aug_matrix'], np.float64)
    inv_pr = np.linalg.inv(aug[..., :3, :3])
    post_trans = aug[..., :3, 3]
    A64 = inv_pr
    b64 = -np.einsum('bnij,bnj->bni', inv_pr, post_trans)
    combine = c2e[..., :3, :3] @ np.linalg.inv(intr[..., :3, :3])
    pre = laug[..., :3, :3] @ np.linalg.inv(l2e[..., :3, :3])
    M64 = np.einsum('bij,bnjk->bnik', pre, combine)
    t64 = np.einsum('bij,bnj->bni', pre, c2e[..., :3, 3] - l2e[..., :3, 3][:, None, :]) \
        + laug[..., :3, 3][:, None, :]
    return A64[0], b64[0], M64[0], t64[0]


def _geom64(A, b, M, t, px, py, dv):
    p0 = [A[k, 0] * px + A[k, 1] * py + (A[k, 2] * dv + b[k]) for k in range(3)]
    uu = p0[0] * p0[2]
    vv = p0[1] * p0[2]
    return [(uu * M[k, 0] + vv * M[k, 1]) + p0[2] * M[k, 2] + t[k] for k in range(3)]


def _build_plan(inputs):
    A64, b64, M64, t64 = _compute_coeffs(inputs)
    ds, xs, ys = _frustum_axes()
    dvg, pyg, pxg = np.meshgrid(ds.astype(np.float64), ys.astype(np.float64),
                                xs.astype(np.float64), indexing='ij')
    pxg = np.ascontiguousarray(pxg.ravel())
    pyg = np.ascontiguousarray(pyg.ravel())
    dvg = np.ascontiguousarray(dvg.ravel())
    cores = [[] for _ in range(NCORES)]
    for n in range(N):
        gx, gy, gz = _geom64(A64[n], b64[n], M64[n], t64[n], pxg, pyg, dvg)
        qx = (gx - COFF[0]) / DXS[0]
        qy = (gy - COFF[1]) / DXS[1]
        qz = (gz - COFF[2]) / DXS[2]
        m = MARGIN_Q
        keep = ((qx > -1 - m) & (qx < NX + m) &
                (qy > -1 - m) & (qy < NY + m) &
                (qz > -1 - m) & (qz < NZ + m))
        idx = np.nonzero(keep)[0]
        kx = np.maximum(np.floor(qx[idx]), 0).astype(np.int64)
        ky = np.maximum(np.floor(qy[idx]), 0).astype(np.int64)
        code = np.zeros(len(kx), np.int64)
        for bit in range(9):
            code |= ((kx >> bit) & 1) << (2 * bit) | ((ky >> bit) & 1) << (2 * bit + 1)
        order = np.argsort(code, kind='stable')
        idx, kx, ky = idx[order], kx[order], ky[order]
        K = len(idx)
        bounds = [K * c // NCORES for c in range(NCORES + 1)]
        for c in range(NCORES):
            lo, hi = bounds[c], bounds[c + 1]
            i = lo
            while i < hi:
                j = i
                x0 = x1 = kx[i]; y0 = y1 = ky[i]
                while j < hi and j - i < BLK:
                    nx0 = min(x0, kx[j]); nx1 = max(x1, kx[j])
                    ny0 = min(y0, ky[j]); ny1 = max(y1, ky[j])
                    if nx1 - nx0 + 3 > WXP or ny1 - ny0 + 3 > WYP:
                        break
                    x0, x1, y0, y1 = nx0, nx1, ny0, ny1
                    j += 1
                cores[c].append(dict(idx=idx[i:j], cam=n,
                                     kx0=max(int(x0) - 1, 0),
                                     ky0=max(int(y0) - 1, 0)))
                i = j
    NBC = max(len(c) for c in cores)
    return dict(A64=A64, b64=b64, M64=M64, t64=t64, cores=cores, NBC=NBC,
                pxg=pxg, pyg=pyg, dvg=dvg)


def _pack_core(plan, inputs, c):
    """Device-side tables for core c."""
    Lx, Ly, Lz = _get_thresholds()
    NBC = plan['NBC']
    cf = np.asarray(inputs['cam_feats'], f32)[0].reshape(N, -1, C)
    blocks = plan['cores'][c]
    feats = np.zeros((NBC, BLK, C), ml_dtypes.bfloat16)
    pxt = np.zeros((128, UJ * NBC), f32)
    pyt = np.zeros((128, UJ * NBC), f32)
    dvt = np.zeros((128, UJ * NBC), f32)
    coef = np.zeros((NBC, NCO), f32)
    thrx = np.full((NBC, WXP + 1), PADTHR, f32)
    thry = np.full((NBC, WYP + 1), PADTHR, f32)
    for s, blk in enumerate(blocks):
        n = blk['cam']
        idx = blk['idx']
        k = len(idx)
        feats[s, :k] = cf[n][idx].astype(ml_dtypes.bfloat16)
        px = np.zeros(BLK, f32); py = np.zeros(BLK, f32); dv = np.zeros(BLK, f32)
        px[:k] = plan['pxg'][idx]; py[:k] = plan['pyg'][idx]; dv[:k] = plan['dvg'][idx]
        pxt[:, s * UJ:(s + 1) * UJ] = px.reshape(128, UJ)
        pyt[:, s * UJ:(s + 1) * UJ] = py.reshape(128, UJ)
        dvt[:, s * UJ:(s + 1) * UJ] = dv.reshape(128, UJ)
        A = plan['A64'][n].astype(f32); b = plan['b64'][n].astype(f32)
        M = plan['M64'][n].astype(f32); t = plan['t64'][n].astype(f32)
        coef[s] = np.array(list(A.ravel()) + list(b) + list(M.ravel()) + list(t), f32)
        ex = min(WXP + 1, NX + 1 - blk['kx0'])
        ey = min(WYP + 1, NY + 1 - blk['ky0'])
        thrx[s, :ex] = Lx[blk['kx0']:blk['kx0'] + ex]
        thry[s, :ey] = Ly[blk['ky0']:blk['ky0'] + ey]
    coefb = np.broadcast_to(coef.reshape(1, NBC * NCO), (128, NBC * NCO)).copy()
    thrxb = np.broadcast_to(thrx.reshape(1, NBC * (WXP + 1)),
                            (128, NBC * (WXP + 1))).copy()
    thryb = np.broadcast_to(thry.reshape(1, NBC * (WYP + 1)),
                            (128, NBC * (WYP + 1))).copy()
    return dict(feats=feats, pxt=pxt, pyt=pyt, dvt=dvt, coef=coefb,
                thrx=thrxb, thry=thryb)


# ----------------------------------------------------------------- bass build
def _build_bass(NBC):
    import concourse.bacc as bacc
    import concourse.mybir as mybir
    import concourse.tile as tile

    SJ = NBC * UJ
    f32t = mybir.dt.float32
    bf16 = mybir.dt.bfloat16
    AL = mybir.AluOpType
    Lx, Ly, Lz = _get_thresholds()
    LZ0, LZ1 = float(Lz[0]), float(Lz[1])

    nc = bacc.Bacc(None, target_bir_lowering=False, num_devices=NCORES)
    feats_t = nc.dram_tensor("feats", [NBC, BLK, C], bf16, kind="ExternalInput")
    pxt_t = nc.dram_tensor("pxt", [128, SJ], f32t, kind="ExternalInput")
    pyt_t = nc.dram_tensor("pyt", [128, SJ], f32t, kind="ExternalInput")
    dvt_t = nc.dram_tensor("dvt", [128, SJ], f32t, kind="ExternalInput")
    coef_t = nc.dram_tensor("coef", [128, NBC * NCO], f32t, kind="ExternalInput")
    thrx_t = nc.dram_tensor("thrx", [128, NBC * (WXP + 1)], f32t, kind="ExternalInput")
    thry_t = nc.dram_tensor("thry", [128, NBC * (WYP + 1)], f32t, kind="ExternalInput")
    slots_t = nc.dram_tensor("slots", [C, NBC * WP], f32t, kind="ExternalOutput")

    with tile.TileContext(nc) as tc:
        with tc.tile_pool(name="tabs", bufs=1) as tp, \
             tc.tile_pool(name="fb", bufs=4) as fp, \
             tc.tile_pool(name="oh", bufs=4) as op_, \
             tc.tile_pool(name="ps", bufs=6, space="PSUM") as pp:

            pxt = tp.tile([128, SJ], f32t); nc.sync.dma_start(pxt[:], pxt_t[:])
            pyt = tp.tile([128, SJ], f32t); nc.sync.dma_start(pyt[:], pyt_t[:])
            dvt = tp.tile([128, SJ], f32t); nc.sync.dma_start(dvt[:], dvt_t[:])
            coef = tp.tile([128, NBC * NCO], f32t); nc.sync.dma_start(coef[:], coef_t[:])
            thrx = tp.tile([128, NBC * (WXP + 1)], f32t)
            nc.sync.dma_start(thrx[:], thrx_t[:])
            thry = tp.tile([128, NBC * (WYP + 1)], f32t)
            nc.sync.dma_start(thry[:], thry_t[:])

            def cslice(kidx):
                ap = coef[:].rearrange("p (s k) -> p s k", k=NCO)[:, :, kidx:kidx + 1]
                return ap.broadcast_to([128, NBC, UJ])

            def g3(ap):
                return ap.rearrange("p (s j) -> p s j", j=UJ)

            # ---- batched geometry, exact f32 op order ----
            tmpa = tp.tile([128, SJ], f32t)
            tmpb = tp.tile([128, SJ], f32t)
            p0 = [tp.tile([128, SJ], f32t, name=f'p0_{i}', tag=f'p0_{i}')
                  for i in range(3)]
            for kk in range(3):
                nc.vector.tensor_tensor(out=g3(tmpa[:]), in0=g3(pxt[:]),
                                        in1=cslice(3 * kk + 0), op=AL.mult)
                nc.vector.tensor_tensor(out=g3(tmpb[:]), in0=g3(pyt[:]),
                                        in1=cslice(3 * kk + 1), op=AL.mult)
                nc.vector.tensor_tensor(out=tmpa[:], in0=tmpa[:], in1=tmpb[:], op=AL.add)
                nc.vector.tensor_tensor(out=g3(tmpb[:]), in0=g3(dvt[:]),
                                        in1=cslice(3 * kk + 2), op=AL.mult)
                nc.vector.tensor_tensor(out=g3(tmpb[:]), in0=g3(tmpb[:]),
                                        in1=cslice(9 + kk), op=AL.add)
                nc.vector.tensor_tensor(out=p0[kk][:], in0=tmpa[:], in1=tmpb[:], op=AL.add)
            uu = tp.tile([128, SJ], f32t)
            vv = tp.tile([128, SJ], f32t)
            nc.vector.tensor_tensor(out=uu[:], in0=p0[0][:], in1=p0[2][:], op=AL.mult)
            nc.vector.tensor_tensor(out=vv[:], in0=p0[1][:], in1=p0[2][:], op=AL.mult)
            g = [tp.tile([128, SJ], f32t, name=f'g_{i}', tag=f'g_{i}') for i in range(3)]
            for kk in range(3):
                base = 12 + 3 * kk
                nc.vector.tensor_tensor(out=g3(tmpa[:]), in0=g3(uu[:]),
                                        in1=cslice(base + 0), op=AL.mult)
                nc.vector.tensor_tensor(out=g3(tmpb[:]), in0=g3(vv[:]),
                                        in1=cslice(base + 1), op=AL.mult)
                nc.vector.tensor_tensor(out=tmpa[:], in0=tmpa[:], in1=tmpb[:], op=AL.add)
                nc.vector.tensor_tensor(out=g3(tmpb[:]), in0=g3(p0[2][:]),
                                        in1=cslice(base + 2), op=AL.mult)
                nc.vector.tensor_tensor(out=tmpa[:], in0=tmpa[:], in1=tmpb[:], op=AL.add)
                nc.vector.tensor_tensor(out=g3(g[kk][:]), in0=g3(tmpa[:]),
                                        in1=cslice(21 + kk), op=AL.add)
            gx, gy, gz = g
            # ---- z-range mask (NZ=1): zm = (gz >= Lz0) * (gz < Lz1) ----
            zm = tp.tile([128, SJ], f32t)
            nc.vector.tensor_scalar(out=tmpa[:], in0=gz[:], scalar1=LZ0,
                                    scalar2=None, op0=AL.is_ge)
            nc.vector.tensor_scalar(out=tmpb[:], in0=gz[:], scalar1=LZ1,
                                    scalar2=None, op0=AL.is_lt)
            nc.vector.tensor_tensor(out=zm[:], in0=tmpa[:], in1=tmpb[:], op=AL.mult)

            gx4 = gx[:].rearrange("p (s j) -> p s j", j=UJ)
            gy4 = gy[:].rearrange("p (s j) -> p s j", j=UJ)
            zm4 = zm[:].rearrange("p (s j) -> p s j", j=UJ)

            # ---- batched exact binning: per-axis one-hot indicators ----
            WX1, WY1 = WXP + 1, WYP + 1
            cxa = tp.tile([128, NBC * UJ * WX1], bf16)
            cxa4 = cxa[:].rearrange("p (s j w) -> p s j w", j=UJ, w=WX1)
            nc.vector.tensor_tensor(
                out=cxa4,
                in0=gx4[:, :, :, None].broadcast_to([128, NBC, UJ, WX1]),
                in1=thrx[:].rearrange("p (s w) -> p s w", w=WX1)[:, :, None, :]
                    .broadcast_to([128, NBC, UJ, WX1]),
                op=AL.is_ge)
            axa = tp.tile([128, NBC * UJ * WXP], bf16)
            axa4 = axa[:].rearrange("p (s j w) -> p s j w", j=UJ, w=WXP)
            nc.vector.tensor_tensor(out=axa4, in0=cxa4[:, :, :, 0:WXP],
                                    in1=cxa4[:, :, :, 1:WX1], op=AL.subtract)
            cya = tp.tile([128, NBC * UJ * WY1], bf16)
            cya4 = cya[:].rearrange("p (s j w) -> p s j w", j=UJ, w=WY1)
            nc.vector.tensor_tensor(
                out=cya4,
                in0=gy4[:, :, :, None].broadcast_to([128, NBC, UJ, WY1]),
                in1=thry[:].rearrange("p (s w) -> p s w", w=WY1)[:, :, None, :]
                    .broadcast_to([128, NBC, UJ, WY1]),
                op=AL.is_ge)
            ayt = tp.tile([128, NBC * UJ * WYP], bf16)
            ayt4 = ayt[:].rearrange("p (s j w) -> p s j w", j=UJ, w=WYP)
            nc.vector.tensor_tensor(out=ayt4, in0=cya4[:, :, :, 0:WYP],
                                    in1=cya4[:, :, :, 1:WY1], op=AL.subtract)
            nc.vector.tensor_tensor(
                out=ayt4, in0=ayt4,
                in1=zm4[:, :, :, None].broadcast_to([128, NBC, UJ, WYP]),
                op=AL.mult)

            slots = tp.tile([C, NBC * WP], f32t)

            # ---- per-block: one-hot outer product + matmul scatter ----
            PAIR = 4
            for s0 in range(0, NBC, PAIR):
                sl = min(PAIR, NBC - s0)
                fb = fp.tile([128, PAIR * UJ * C], bf16, tag="fb", name="fb")
                nc.sync.dma_start(
                    fb[:, :sl * UJ * C].rearrange("p (s x) -> p s x", x=UJ * C),
                    feats_t[s0:s0 + sl].rearrange("s (p j) c -> p s (j c)", p=128))
                for si in range(sl):
                    s = s0 + si
                    oh = op_.tile([128, UJ * WP], bf16, tag="oh", name="oh")
                    oh4 = oh[:].rearrange("p (j y x) -> p j y x", y=WYP, x=WXP)
                    nc.vector.tensor_tensor(
                        out=oh4,
                        in0=ayt4[:, s][:, :, :, None].broadcast_to([128, UJ, WYP, WXP]),
                        in1=axa4[:, s][:, :, None, :].broadcast_to([128, UJ, WYP, WXP]),
                        op=AL.mult)
                    ps = pp.tile([C, WP], mybir.dt.float32, space="PSUM", tag="ps",
                                 name="ps")
                    for j in range(UJ):
                        nc.tensor.matmul(
                            ps[:],
                            lhsT=fb[:, (si * UJ + j) * C:(si * UJ + j + 1) * C],
                            rhs=oh[:, j * WP:(j + 1) * WP],
                            start=(j == 0), stop=(j == UJ - 1))
                    nc.vector.tensor_copy(out=slots[:, s * WP:(s + 1) * WP], in_=ps[:])

            nc.sync.dma_start(slots_t[:], slots[:])

    nc.compile()
    return nc


_CACHE = {}


def kernel(**inputs) -> np.ndarray:
    from concourse.bass_utils import run_bass_kernel_spmd

    plan = _build_plan(inputs)
    NBC = plan['NBC']
    if NBC == 0:
        return np.zeros((B, C, NX, NY), f32)
    in_maps = [_pack_core(plan, inputs, c) for c in range(NCORES)]
    if NBC not in _CACHE:
        _CACHE.clear()
        _CACHE[NBC] = _build_bass(NBC)
    nc = _CACHE[NBC]

    r = run_bass_kernel_spmd(nc, in_maps, core_ids=list(range(NCORES)))
    out = np.zeros((B, C, NX, NY), f32)
    for c in range(NCORES):
        slots = r.results[c]['slots'].reshape(C, NBC, WYP, WXP)
        for s, blk in enumerate(plan['cores'][c]):
            kx0, ky0 = blk['kx0'], blk['ky0']
            ex = min(WXP, NX - kx0); ey = min(WYP, NY - ky0)
            # slot layout [C, y, x]; output layout [C, X, Y]
            out[0, :, kx0:kx0 + ex, ky0:ky0 + ey] += \
                slots[:, s, :ey, :ex].transpose(0, 2, 1)
    return out
